# revision 13
# baseline (speedup 1.0000x reference)
"""CustomKMeansAttention Trainium2 kernel.

Strategy (8 NeuronCores, SPMD, no collectives):
  Launch 1 (core c, b=c//2, j=c%2):
    - q,v projection for own 2048 rows (all heads), k projection for the
      full batch b but only heads j*8..j*8+8 (head-split between the two
      cores of a batch) -> PE matmuls.
    - full 10-iter kmeans over (4096 keys x 8 clusters) for its 8 (b,h)
      pairs on device (PE distance/centroid matmuls + DVE argmin one-hot),
      final per-key distance to assigned centroid -> kd.
  Host: jax PRNG constants (centroid init indices, Gumbel noise) which are
    input-independent, probs + Gumbel top-64 index selection, tiny gathers.
  Launch 2 (row-parallel): attention of 2048 rows x 16 heads against the
    64 sampled keys/values + output projection.
"""

import sys

sys.path.insert(0, "/opt/trn_rl_repo")

import numpy as np

import concourse.bass as bass
import concourse.bacc as bacc
import concourse.mybir as mybir
from concourse import tile, masks
from concourse.bass_utils import run_bass_kernel_spmd

F32 = mybir.dt.float32
AF = mybir.ActivationFunctionType
ALU = mybir.AluOpType
AX = mybir.AxisListType

B, N, C = 4, 4096, 1024
H, Dh = 16, 64
NCl, NS, NIT = 8, 64, 10
K = N
RPC = N // 2          # rows per core (2048)
HPC = H // 2          # kmeans heads per core (8)
BIG = 1.0e4
SCALE = Dh ** -0.5


def build_launch1():
    nc = bacc.Bacc("TRN2", target_bir_lowering=False, debug=False)
    xT_full = nc.dram_tensor("xT_full", [C, N], F32, kind="ExternalInput").ap()
    xT_own = nc.dram_tensor("xT_own", [C, RPC], F32, kind="ExternalInput").ap()
    WqvT = nc.dram_tensor("WqvT", [C, 2 * C], F32, kind="ExternalInput").ap()
    WkT = nc.dram_tensor("WkT", [C, HPC * Dh], F32, kind="ExternalInput").ap()
    bqv = nc.dram_tensor("bqv", [2 * C, 1], F32, kind="ExternalInput").ap()
    bk = nc.dram_tensor("bk", [HPC * Dh, 1], F32, kind="ExternalInput").ap()
    cinit = nc.dram_tensor("cinit", [HPC, NCl, 65], F32, kind="ExternalInput").ap()
    iotaB = nc.dram_tensor("iotaB", [128, 256], F32, kind="ExternalInput").ap()
    iota2 = nc.dram_tensor("iota2", [128, 256], F32, kind="ExternalInput").ap()

    qT = nc.dram_tensor("qT", [C, RPC], F32, kind="ExternalOutput").ap()
    vT = nc.dram_tensor("vT", [C, RPC], F32, kind="ExternalOutput").ap()
    kTo = nc.dram_tensor("kTo", [HPC * Dh, N], F32, kind="ExternalOutput").ap()
    kd = nc.dram_tensor("kd", [HPC, 128, 32], F32, kind="ExternalOutput").ap()

    with tile.TileContext(nc) as tc:
        with tc.tile_pool(name="const", bufs=1) as cpool:
            ident = cpool.tile([128, 128], F32, tag="ident")
            masks.make_identity(nc, ident[:])
            scol = cpool.tile([65, 1], F32, tag="scol")
            nc.gpsimd.memset(scol[0:64, :], -2.0)
            nc.gpsimd.memset(scol[64:65, :], 1.0)
            iob = cpool.tile([128, 256], F32, tag="iob")
            nc.sync.dma_start(iob[:], iotaB[:])
            io2 = cpool.tile([128, 256], F32, tag="io2")
            nc.sync.dma_start(io2[:], iota2[:])

            # resident weights; bounce through a DVE copy so matmuls reading
            # them depend on the DVE semaphore, not a second DMA queue
            # (instructions support at most ~2 waits / 1 DMA-queue wait).
            with tc.tile_pool(name="wstg", bufs=2) as wsp:
                wqv = []
                for cc in range(8):
                    s = wsp.tile([128, 2 * C], F32, tag="wstg")
                    nc.sync.dma_start(s[:], WqvT[cc * 128:(cc + 1) * 128, :])
                    t = cpool.tile([128, 2 * C], F32, tag=f"wqv{cc}")
                    nc.vector.tensor_copy(t[:], s[:])
                    wqv.append(t)
                wk = []
                for cc in range(8):
                    s = wsp.tile([128, 2 * C], F32, tag="wstg")
                    nc.sync.dma_start(
                        s[:, 0:HPC * Dh], WkT[cc * 128:(cc + 1) * 128, :])
                    t = cpool.tile([128, HPC * Dh], F32, tag=f"wk{cc}")
                    nc.vector.tensor_copy(t[:], s[:, 0:HPC * Dh])
                    wk.append(t)
            bqvc = []
            for ot in range(16):
                t = cpool.tile([128, 1], F32, tag=f"bqv{ot}")
                nc.sync.dma_start(t[:], bqv[ot * 128:(ot + 1) * 128, :])
                bqvc.append(t)
            bkc = []
            for ot in range(4):
                t = cpool.tile([128, 1], F32, tag=f"bk{ot}")
                nc.sync.dma_start(t[:], bk[ot * 128:(ot + 1) * 128, :])
                bkc.append(t)

            # ---------------- phase A: q,v for own rows -----------------
            with tc.tile_pool(name="xa", bufs=16) as xpool, \
                 tc.tile_pool(name="eva", bufs=4) as evpool, \
                 tc.tile_pool(name="psA", bufs=4, space="PSUM") as psA:
                for rs in range(RPC // 512):
                    xs = []
                    for cc in range(8):
                        t = xpool.tile([128, 512], F32, tag="x")
                        nc.sync.dma_start(
                            t[:], xT_own[cc * 128:(cc + 1) * 128,
                                         rs * 512:(rs + 1) * 512])
                        xs.append(t)
                    for ot in range(16):
                        ps = psA.tile([128, 512], F32, tag="ps")
                        for cc in range(8):
                            nc.tensor.matmul(
                                ps[:], wqv[cc][:, ot * 128:(ot + 1) * 128],
                                xs[cc][:], start=(cc == 0), stop=(cc == 7))
                        ev = evpool.tile([128, 512], F32, tag="ev")
                        nc.vector.tensor_scalar_add(ev[:], ps[:], bqvc[ot][:])
                        dst = qT if ot < 8 else vT
                        col = (ot * 128) if ot < 8 else ((ot - 8) * 128)
                        nc.sync.dma_start(
                            dst[col:col + 128, rs * 512:(rs + 1) * 512], ev[:])

                # ---------------- phase B: k for full batch -----------------
                for rs in range(N // 512):
                    xs = []
                    for cc in range(8):
                        t = xpool.tile([128, 512], F32, tag="x")
                        nc.sync.dma_start(
                            t[:], xT_full[cc * 128:(cc + 1) * 128,
                                          rs * 512:(rs + 1) * 512])
                        xs.append(t)
                    for ot in range(4):
                        ps = psA.tile([128, 512], F32, tag="ps")
                        for cc in range(8):
                            nc.tensor.matmul(
                                ps[:], wk[cc][:, ot * 128:(ot + 1) * 128],
                                xs[cc][:], start=(cc == 0), stop=(cc == 7))
                        ev = evpool.tile([128, 512], F32, tag="ev")
                        nc.vector.tensor_scalar_add(ev[:], ps[:], bkc[ot][:])
                        nc.sync.dma_start(
                            kTo[ot * 128:(ot + 1) * 128,
                                rs * 512:(rs + 1) * 512], ev[:])

            # ---------------- phase C: kmeans per head -----------------
            with tc.tile_pool(name="kta", bufs=2) as ktp, \
                 tc.tile_pool(name="kra", bufs=2) as krp, \
                 tc.tile_pool(name="sqp", bufs=2) as sqp, \
                 tc.tile_pool(name="smal", bufs=4) as smp, \
                 tc.tile_pool(name="crap", bufs=3) as crp, \
                 tc.tile_pool(name="big", bufs=4) as bgp, \
                 tc.tile_pool(name="tp_ps", bufs=2, space="PSUM") as tpps, \
                 tc.tile_pool(name="ct_ps", bufs=1, space="PSUM") as ctps, \
                 tc.tile_pool(name="sc_ps", bufs=2, space="PSUM") as scps, \
                 tc.tile_pool(name="cs_ps", bufs=2, space="PSUM") as csps:
                for h in range(HPC):
                    kTa = ktp.tile([65, N], F32, tag="kta")
                    nc.sync.dma_start(kTa[0:64, :], kTo[h * 64:(h + 1) * 64, :])
                    nc.gpsimd.memset(kTa[64:65, :], 1.0)
                    kra = krp.tile([128, 32 * 65], F32, tag="kra")
                    nc.gpsimd.memset(kra[:], 1.0)
                    for kt in range(32):
                        tp = tpps.tile([128, 64], F32, tag="tp")
                        nc.tensor.transpose(
                            tp[:], kTa[0:64, kt * 128:(kt + 1) * 128],
                            ident[0:64, 0:64])
                        nc.vector.tensor_copy(
                            kra[:, kt * 65:kt * 65 + 64], tp[:])
                    kview = kra[:].rearrange("p (t c) -> p t c", c=65)[:, :, 0:64]
                    sq = sqp.tile([128, 2048], F32, tag="sq")
                    sqv = sq[:].rearrange("p (t c) -> p t c", c=64)
                    nc.vector.tensor_tensor(sqv, kview, kview, op=ALU.mult)
                    kk = smp.tile([128, 32], F32, tag="kk")
                    nc.vector.tensor_reduce(kk[:], sqv, axis=AX.X, op=ALU.add)

                    cra = crp.tile([NCl, 65], F32, tag="cra")
                    nc.sync.dma_start(cra[:], cinit[h])

                    for it in range(NIT + 1):
                        ctp = ctps.tile([65, NCl], F32, tag="ctp")
                        nc.tensor.transpose(ctp[:], cra[:], ident[0:NCl, 0:NCl])
                        ctn = smp.tile([65, NCl], F32, tag="ctn")
                        nc.vector.tensor_scalar_mul(ctn[:], ctp[:], scol[:])
                        sc = scps.tile([128, 256], F32, tag="sc")
                        for kt in range(32):
                            nc.tensor.matmul(
                                sc[:, kt * 8:(kt + 1) * 8],
                                kTa[:, kt * 128:(kt + 1) * 128], ctn[:],
                                start=True, stop=True)
                        scv = sc[:].rearrange("p (t c) -> p t c", c=8)
                        m = smp.tile([128, 32], F32, tag="m")
                        nc.vector.tensor_reduce(m[:], scv, axis=AX.X, op=ALU.min)
                        mb = m[:].rearrange("p (t o) -> p t o", o=1) \
                            .broadcast_to([128, 32, 8])
                        if it < NIT:
                            eq = bgp.tile([128, 256], F32, tag="eq")
                            eqv = eq[:].rearrange("p (t c) -> p t c", c=8)
                            nc.vector.tensor_tensor(eqv, scv, mb, op=ALU.is_equal)
                            nc.vector.tensor_tensor(
                                eq[:], eq[:], iob[:], op=ALU.mult)
                            nc.vector.tensor_scalar_add(eq[:], eq[:], BIG)
                            idx = smp.tile([128, 32], F32, tag="idx")
                            nc.vector.tensor_reduce(
                                idx[:], eqv, axis=AX.X, op=ALU.min)
                            idxb = idx[:].rearrange("p (t o) -> p t o", o=1) \
                                .broadcast_to([128, 32, 8])
                            oh = bgp.tile([128, 256], F32, tag="oh")
                            ohv = oh[:].rearrange("p (t c) -> p t c", c=8)
                            nc.vector.tensor_tensor(
                                ohv, io2[:].rearrange("p (t c) -> p t c", c=8),
                                idxb, op=ALU.is_equal)
                            cs = csps.tile([NCl, 65], F32, tag="cs")
                            for kt in range(32):
                                nc.tensor.matmul(
                                    cs[:], oh[:, kt * 8:(kt + 1) * 8],
                                    kra[:, kt * 65:(kt + 1) * 65],
                                    start=(kt == 0), stop=(kt == 31))
                            den = smp.tile([NCl, 1], F32, tag="den")
                            nc.vector.tensor_scalar_add(
                                den[:], cs[:, 64:65], 1e-6)
                            rp = smp.tile([NCl, 1], F32, tag="rp")
                            nc.vector.reciprocal(rp[:], den[:])
                            cnew = smp.tile([NCl, 64], F32, tag="cnew")
                            nc.vector.tensor_scalar_mul(
                                cnew[:], cs[:, 0:64], rp[:])
                            msk = smp.tile([NCl, 1], F32, tag="msk")
                            nc.vector.tensor_scalar_min(
                                msk[:], cs[:, 64:65], 1.0)
                            dd = smp.tile([NCl, 64], F32, tag="dd")
                            nc.vector.tensor_tensor(
                                dd[:], cnew[:], cra[:, 0:64], op=ALU.subtract)
                            nc.vector.tensor_scalar_mul(dd[:], dd[:], msk[:])
                            cra2 = crp.tile([NCl, 65], F32, tag="cra")
                            nc.vector.tensor_tensor(
                                cra2[:, 0:64], cra[:, 0:64], dd[:], op=ALU.add)
                            sqc = smp.tile([NCl, 64], F32, tag="sqc")
                            nc.vector.tensor_tensor(
                                sqc[:], cra2[:, 0:64], cra2[:, 0:64],
                                op=ALU.mult)
                            nc.vector.tensor_reduce(
                                cra2[:, 64:65], sqc[:], axis=AX.X, op=ALU.add)
                            cra = cra2
                        else:
                            kdt = smp.tile([128, 32], F32, tag="kdt")
                            nc.vector.tensor_tensor(
                                kdt[:], m[:], kk[:], op=ALU.add)
                            nc.vector.tensor_scalar_max(kdt[:], kdt[:], 0.0)
                            kds = smp.tile([128, 32], F32, tag="kds")
                            nc.scalar.activation(kds[:], kdt[:], AF.Sqrt)
                            nc.sync.dma_start(kd[h], kds[:])
    nc.compile()
    return nc


def build_launch2():
    nc = bacc.Bacc("TRN2", target_bir_lowering=False, debug=False)
    qTi = nc.dram_tensor("qTi", [C, RPC], F32, kind="ExternalInput").ap()
    tk = nc.dram_tensor("tk", [H, Dh, NS], F32, kind="ExternalInput").ap()
    tv = nc.dram_tensor("tv", [H, NS, Dh], F32, kind="ExternalInput").ap()
    WpT = nc.dram_tensor("WpT", [C, C], F32, kind="ExternalInput").ap()
    bp = nc.dram_tensor("bp", [1, C], F32, kind="ExternalInput").ap()
    outr = nc.dram_tensor("outr", [RPC, C], F32, kind="ExternalOutput").ap()

    with tile.TileContext(nc) as tc:
        with tc.tile_pool(name="const", bufs=1) as cpool:
            ident = cpool.tile([128, 128], F32, tag="ident")
            masks.make_identity(nc, ident[:])
            ones = cpool.tile([1, 128], F32, tag="ones")
            nc.gpsimd.memset(ones[:], 1.0)
            bpt = cpool.tile([1, C], F32, tag="bpt")
            nc.sync.dma_start(bpt[:], bp[:])
            wp = []
            for g in range(8):
                t = cpool.tile([128, C], F32, tag=f"wp{g}")
                nc.sync.dma_start(t[:], WpT[g * 128:(g + 1) * 128, :])
                wp.append(t)
            # Bounce sampled K/V through a DVE copy: PE matmuls reading them
            # then wait on the DVE semaphore (shared with other operands)
            # instead of a second DMA-queue semaphore — the PE instruction
            # wait-slot limit is 2.
            tkt = []
            tvt = []
            stg = cpool.tile([128, (H // 2) * NS], F32, tag="stgk")
            nc.sync.dma_start(
                stg[0:64, :].rearrange("d (h s) -> d h s", s=NS),
                tk[0:H:2].rearrange("h d s -> d h s"))
            nc.sync.dma_start(
                stg[64:128, :].rearrange("d (h s) -> d h s", s=NS),
                tk[1:H:2].rearrange("h d s -> d h s"))
            stv = cpool.tile([NS, H * Dh], F32, tag="stgv")
            nc.sync.dma_start(
                stv[:].rearrange("s (h d) -> s h d", d=Dh),
                tv[:].rearrange("h s d -> s h d"))
            for g in range(8):
                t = cpool.tile([128, NS], F32, tag=f"tk{g}")
                # split halves: each half was written by a single DMA
                nc.vector.tensor_copy(
                    t[0:64, :], stg[0:64, g * NS:(g + 1) * NS])
                nc.vector.tensor_copy(
                    t[64:128, :], stg[64:128, g * NS:(g + 1) * NS])
                tkt.append(t)
            for h in range(H):
                t2 = cpool.tile([NS, Dh], F32, tag=f"tv{h}")
                nc.vector.tensor_copy(t2[:], stv[:, h * Dh:(h + 1) * Dh])
                tvt.append(t2)

            with tc.tile_pool(name="qx", bufs=16) as qxp, \
                 tc.tile_pool(name="att", bufs=2) as atp, \
                 tc.tile_pool(name="sm", bufs=6) as smp, \
                 tc.tile_pool(name="pb", bufs=4) as pbp, \
                 tc.tile_pool(name="lg_ps", bufs=2, space="PSUM") as lgps, \
                 tc.tile_pool(name="pt_ps", bufs=2, space="PSUM") as ptps, \
                 tc.tile_pool(name="vo_ps", bufs=2, space="PSUM") as vops, \
                 tc.tile_pool(name="pj_ps", bufs=2, space="PSUM") as pjps:
                for rt in range(RPC // 128):
                    qx = []
                    for cc in range(8):
                        t = qxp.tile([128, 128], F32, tag="qx")
                        nc.sync.dma_start(
                            t[:], qTi[cc * 128:(cc + 1) * 128,
                                      rt * 128:(rt + 1) * 128])
                        qx.append(t)
                    attT = atp.tile([128, 1024], F32, tag="attT")
                    for h in range(H):
                        g, po = h // 2, (h % 2) * 64
                        lg = lgps.tile([128, NS], F32, tag="lg")
                        nc.tensor.matmul(
                            lg[:], qx[g][po:po + 64, :], tkt[g][po:po + 64, :],
                            start=True, stop=True)
                        # exp without max-subtraction: logits here are O(5),
                        # fp32 exp is safe and softmax is shift-invariant.
                        pb = pbp.tile([128, NS], F32, tag="pb")
                        rsum = smp.tile([128, 1], F32, tag="rsum")
                        nc.scalar.activation(
                            pb[:], lg[:], AF.Exp, scale=SCALE,
                            accum_out=rsum[:])
                        rp = smp.tile([128, 1], F32, tag="rp")
                        nc.vector.reciprocal(rp[:], rsum[:])
                        # normalize on ACT so pb has a single writer engine
                        pbn = pbp.tile([128, NS], F32, tag="pbn")
                        nc.scalar.activation(
                            pbn[:], pb[:], AF.Copy, scale=rp[:])
                        pt = ptps.tile([64, 128], F32, tag="pt")
                        nc.tensor.transpose(pt[:], pbn[:], ident[:])
                        pts = pbp.tile([64, 128], F32, tag="pts")
                        nc.vector.tensor_copy(pts[:], pt[:])
                        vo = vops.tile([64, 128], F32, tag="vo")
                        nc.tensor.matmul(
                            vo[:], tvt[h][:], pts[:],
                            start=True, stop=True)
                        nc.vector.tensor_copy(
                            attT[po:po + 64, g * 128:(g + 1) * 128], vo[:])
                    for ct in range(2):
                        pj = pjps.tile([128, 512], F32, tag="pj")
                        for g in range(8):
                            nc.tensor.matmul(
                                pj[:], attT[:, g * 128:(g + 1) * 128],
                                wp[g][:, ct * 512:(ct + 1) * 512],
                                start=(g == 0), stop=False)
                        nc.tensor.matmul(
                            pj[:], ones[:], bpt[:, ct * 512:(ct + 1) * 512],
                            start=False, stop=True)
                        ev = pbp.tile([128, 512], F32, tag="evo")
                        nc.vector.tensor_copy(ev[:], pj[:])
                        nc.sync.dma_start(
                            outr[rt * 128:(rt + 1) * 128,
                                 ct * 512:(ct + 1) * 512], ev[:])
    nc.compile()
    return nc


def _host_consts():
    import jax

    with jax.default_device(jax.devices("cpu")[0]):
        rng = jax.random.key(42)
        k_init, k_gumbel = jax.random.split(rng)
        rand_idx = np.asarray(jax.random.randint(k_init, (B, H, NCl), 0, K))
        g = np.asarray(
            jax.random.gumbel(k_gumbel, (B, H, K), dtype=np.float32))
    return rand_idx, g


_iotaB = np.tile((np.arange(8, dtype=np.float32) - BIG), 32)[None, :].repeat(
    128, 0).copy()
_iota2 = np.tile(np.arange(8, dtype=np.float32), 32)[None, :].repeat(
    128, 0).copy()


def kernel(x, W_qkv, b_qkv, W_proj, b_proj):
    x = np.asarray(x, np.float32)
    W_qkv = np.asarray(W_qkv, np.float32)
    b_qkv = np.asarray(b_qkv, np.float32)
    W_proj = np.asarray(W_proj, np.float32)
    b_proj = np.asarray(b_proj, np.float32)

    rand_idx, gum = _host_consts()

    Wq, Wk, Wv = W_qkv[0:C], W_qkv[C:2 * C], W_qkv[2 * C:3 * C]
    bq, bkk, bv = b_qkv[0:C], b_qkv[C:2 * C], b_qkv[2 * C:3 * C]
    WqvT = np.ascontiguousarray(np.concatenate([Wq, Wv], 0).T)
    bqv = np.concatenate([bq, bv], 0)[:, None].copy()

    # ---- launch 1 ----
    nc1 = build_launch1()
    in_maps = []
    for c in range(8):
        b, j = c // 2, c % 2
        hb = j * HPC
        xT_full = np.ascontiguousarray(x[b].T)
        xT_own = np.ascontiguousarray(x[b, j * RPC:(j + 1) * RPC].T)
        WkTc = np.ascontiguousarray(Wk[hb * Dh:(hb + HPC) * Dh].T)
        bkc = bkk[hb * Dh:(hb + HPC) * Dh][:, None].copy()
        cin = np.zeros((HPC, NCl, 65), np.float32)
        for i in range(HPC):
            hg = hb + i
            rows = x[b, rand_idx[b, hg]]                      # (8, C)
            cvals = (rows @ Wk[hg * Dh:(hg + 1) * Dh].T
                     + bkk[hg * Dh:(hg + 1) * Dh]).astype(np.float32)
            cin[i, :, 0:64] = cvals
            cin[i, :, 64] = (cvals.astype(np.float32) ** 2).sum(-1)
        in_maps.append({
            "xT_full": xT_full, "xT_own": xT_own, "WqvT": WqvT,
            "WkT": WkTc, "bqv": bqv, "bk": bkc, "cinit": cin,
            "iotaB": _iotaB, "iota2": _iota2,
        })
    res1 = run_bass_kernel_spmd(nc1, in_maps, list(range(8))).results

    # ---- host: sampling ----
    kd = np.zeros((B, H, K), np.float32)
    for c in range(8):
        b, j = c // 2, c % 2
        for i in range(HPC):
            kd[b, j * HPC + i] = res1[c]["kd"][i].T.reshape(K)
    probs = kd / (kd.sum(-1, keepdims=True) + 1e-6)
    scores = np.log(probs + 1e-20) + gum
    idx = np.argsort(-scores, axis=-1)[..., :NS]              # (B,H,NS)

    # ---- launch 2 ----
    nc2 = build_launch2()
    WpT = np.ascontiguousarray(W_proj.T)
    bpr = b_proj[None, :].copy()
    in_maps2 = []
    for c in range(8):
        b, j = c // 2, c % 2
        kT_b = np.concatenate([res1[2 * b]["kTo"], res1[2 * b + 1]["kTo"]], 0)
        vT_b = np.concatenate(
            [res1[2 * b]["vT"], res1[2 * b + 1]["vT"]], 1)    # (C, N)
        tk = np.zeros((H, Dh, NS), np.float32)
        tv = np.zeros((H, NS, Dh), np.float32)
        for hg in range(H):
            ii = idx[b, hg]
            tk[hg] = kT_b[hg * Dh:(hg + 1) * Dh][:, ii]
            tv[hg] = vT_b[hg * Dh:(hg + 1) * Dh][:, ii].T
        in_maps2.append({
            "qTi": res1[c]["qT"], "tk": tk, "tv": tv, "WpT": WpT, "bp": bpr,
        })
    res2 = run_bass_kernel_spmd(nc2, in_maps2, list(range(8))).results

    out = np.zeros((B, N, C), np.float32)
    for c in range(8):
        b, j = c // 2, c % 2
        out[b, j * RPC:(j + 1) * RPC] = res2[c]["outr"]
    return out
